# revision 23
# baseline (speedup 1.0000x reference)
"""AttentionProtoNet pooling kernel for 8x TRN2 NeuronCores.

reference (per sample of B=64, L=512, H=768):
    upsilon = tanh(hs @ W_fc.T + b_fc)        [L, H]
    nu      = upsilon @ W_nu                  [L]
    alphas  = softmax(nu)                     [L]
    pooled  = alphas @ hs                     [H]

Strategy: data-parallel over B (8 samples per core), everything on the wire
in fp16 (1 cycle/row on the PE like bf16, but with 10 mantissa bits, and a
single X^T copy feeds both the TensorEngine matmul and the VectorEngine
pooling). The PE runs back-to-back 512-row fp16 matmuls at its 216 ns
roofline cadence; each sample's nu/softmax/pooling epilogue is emitted
inside the NEXT sample's matmul stream so the PE never waits on ACT/DVE,
and the output drain (PE transpose -> copy -> DRAM) trails two samples
behind. DMA goes through the sync HW queue ordered so the PE starts as
early as possible (k0/k1 weights, first sample in halves, k2, remaining
weights, remaining samples). The last sample is processed in two 256-token
halves so most of its softmax/pooling chain overlaps its own matmuls.
"""

import sys

sys.path.insert(0, "/opt/trn_rl_repo")

import numpy as np

B, L, H = 64, 512, 768
NCORES = 8
SPC = B // NCORES            # samples per core
TOK = SPC * L                # tokens per core
HC = H // 128                # 128-partition chunks of H
HL = L // 2                  # token half for the last sample
WARMUP_MM = 9                # junk matmuls: p-state + HAM ramp during DMA

_compiled = {}


def _build():
    import concourse.bass as bass
    import concourse.bacc as bacc
    import concourse.tile as tile
    from concourse import mybir
    from concourse.masks import make_identity

    F32 = mybir.dt.float32
    F16 = mybir.dt.float16
    AF = mybir.ActivationFunctionType
    ALU = mybir.AluOpType

    nc = bacc.Bacc(None, target_bir_lowering=False)

    xt_d = nc.dram_tensor("xt", [128, SPC * HC * L], F16, kind="ExternalInput")
    # weights packed k-strip-major so each weight DMA is 128 large
    # contiguous descriptors
    wt0_d = nc.dram_tensor("wt0", [128, 2, HC, 128], F16, kind="ExternalInput")
    wt2_d = nc.dram_tensor("wt2", [128, 1, HC, 128], F16, kind="ExternalInput")
    wtr_d = nc.dram_tensor("wtr", [128, 3, HC, 128], F16, kind="ExternalInput")
    bfc_d = nc.dram_tensor("bfc", [128, HC], F32, kind="ExternalInput")
    wnu_d = nc.dram_tensor("wnu", [128, HC], F16, kind="ExternalInput")
    out_d = nc.dram_tensor("out", [SPC, H], F32, kind="ExternalOutput")

    with tile.TileContext(nc) as tc:
        with tc.tile_pool(name="xp", bufs=1) as xp, \
             tc.tile_pool(name="wp", bufs=1) as wp, \
             tc.tile_pool(name="cst", bufs=1) as cst, \
             tc.tile_pool(name="ups", bufs=2) as upsp, \
             tc.tile_pool(name="sm", bufs=2) as smp, \
             tc.tile_pool(name="outp", bufs=2) as outp, \
             tc.tile_pool(name="mmps", bufs=4, space="PSUM") as mmps, \
             tc.tile_pool(name="nups", bufs=2, space="PSUM") as nups, \
             tc.tile_pool(name="tps", bufs=2, space="PSUM") as tps:

            # ---- PE warmup: junk matmuls with no DMA dependency ramp the
            # p-state and the HAM activity window while the first tiles
            # stream in.
            wu_sb = cst.tile([128, L], F16)
            nc.vector.memset(wu_sb[:], 1.0)
            wu_ps = tps.tile([128, L], F32, tag="tp", name="wu_ps")
            for i in range(WARMUP_MM):
                nc.tensor.matmul(wu_ps[:], wu_sb[:, 0:128], wu_sb[:],
                                 start=(i == 0), stop=(i == WARMUP_MM - 1))

            # ---- DMA: tiny constants on the gpsimd direct queue; weights
            # and X^T through the sync HW queue, interleaved so the first
            # sample's matmuls start as early as possible.
            bfc_sb = cst.tile([128, HC], F32)
            wnu_sb = cst.tile([128, HC], F16)
            wt_sb = wp.tile([128, HC, HC, 128], F16)   # [p, kstrip, h, m]
            xt_sb = xp.tile([128, SPC * HC * L], F16)
            ident = cst.tile([128, 128], F32)

            nc.gpsimd.dma_start(bfc_sb[:], bfc_d[:])
            nc.gpsimd.dma_start(wnu_sb[:], wnu_d[:])

            def xt_sl(s, h):
                return xt_sb[:, (s * HC + h) * L:(s * HC + h + 1) * L]

            nc.sync.dma_start(wt_sb[:, 0:2, :, :], wt0_d[:])
            nc.sync.dma_start(xt_sb[:, 0:3 * L], xt_d[:, 0:3 * L])
            nc.sync.dma_start(xt_sb[:, 3 * L:HC * L], xt_d[:, 3 * L:HC * L])
            nc.sync.dma_start(wt_sb[:, 2:3, :, :], wt2_d[:])
            nc.sync.dma_start(wt_sb[:, 3:6, :, :], wtr_d[:])
            for s in range(1, SPC):
                nc.sync.dma_start(xt_sb[:, s * HC * L:(s + 1) * HC * L],
                                  xt_d[:, s * HC * L:(s + 1) * HC * L])
            make_identity(nc, ident[:])

            # ---- per-sample state carried to later emission points
            ups_t = [None] * SPC
            pu_t = [None] * SPC

            def emit_group(s, k, ups, c0, c1):
                """one k-chunk matmul group + tanh for tokens [c0, c1)."""
                ps = mmps.tile([128, L], F32, tag="mm")
                w = c1 - c0
                for h in range(HC):
                    nc.tensor.matmul(
                        ps[:, 0:w],
                        wt_sb[:, k, h, :],
                        xt_sl(s, h)[:, c0:c1],
                        start=(h == 0),
                        stop=(h == HC - 1),
                    )
                nc.scalar.activation(
                    ups[:, k, c0:c1], ps[:, 0:w], AF.Tanh,
                    bias=bfc_sb[:, k:k + 1],
                )

            def emit_epilogue(s):
                """nu + softmax + pooling for sample s, emitted inside a
                later sample's matmul stream where all inputs are done."""
                ups = ups_t[s]
                nu = nups.tile([1, L], F32, tag="nu", name="nu_p")
                for k in range(HC):
                    nc.tensor.matmul(
                        nu[:], wnu_sb[:, k:k + 1], ups[:, k, :],
                        start=(k == 0), stop=(k == HC - 1),
                    )
                # nu is small enough that exp() needs no max subtraction
                ex = smp.tile([1, L], F16, tag="ex")
                z = smp.tile([1, 1], F32, tag="z")
                rz = smp.tile([1, 1], F32, tag="rz")
                nc.scalar.activation(ex[:], nu[:], AF.Exp, accum_out=z[:])
                nc.vector.reciprocal(rz[:], z[:])
                ab = smp.tile([128, L], F16, tag="ab")
                nc.gpsimd.partition_broadcast(ab[:], ex[:])
                rzb = smp.tile([128, 1], F32, tag="rzb")
                nc.gpsimd.partition_broadcast(rzb[:], rz[:])
                # weighted-sum pooling on the VectorEngine; the 1/Z
                # normalization rides the STT per-partition scalar
                pu = outp.tile([128, HC], F32, tag="pu")
                for h in range(HC):
                    trash = smp.tile([128, L], F16, tag="trash")
                    nc.vector.scalar_tensor_tensor(
                        trash[:], xt_sl(s, h), rzb[:, 0:1], ab[:],
                        ALU.mult, ALU.mult,
                        accum_out=pu[:, h:h + 1],
                    )
                pu_t[s] = pu

            def emit_drain(s):
                """pooled^T [128, HC] -> [HC, 128] -> DRAM row s."""
                tp_ps = tps.tile([HC, 128], F32, tag="tp")
                nc.tensor.transpose(tp_ps[:], pu_t[s][:], ident[:])
                orow = outp.tile([HC, 128], F32, tag="orow")
                nc.scalar.copy(orow[:], tp_ps[:])
                nc.sync.dma_start(
                    out_d[s:s + 1, :].rearrange("o (c p) -> (o c) p", p=128),
                    orow[:],
                )

            # ---- samples 0..6: full-width pipeline
            for s in range(SPC - 1):
                ups = upsp.tile([128, HC, L], F16, tag="ups")
                ups_t[s] = ups
                for ji in range(HC):
                    emit_group(s, ji, ups, 0, L)
                    if s > 0 and ji == 1:
                        emit_epilogue(s - 1)
                    if s > 1 and ji == 4:
                        emit_drain(s - 2)

            # ---- last sample: two 256-token halves so the softmax/pool
            # chain of half 0 overlaps half 1's matmuls, and only a short
            # chain trails the final matmul
            s = SPC - 1
            ups = upsp.tile([128, HC, L], F16, tag="ups", name="ups_last")
            ups_t[s] = ups
            korder0 = list(range(HC))
            korder1 = [5, 0, 1, 2, 3, 4]
            nu_a = nups.tile([1, L], F32, tag="nu", name="nu_a")
            nu_b = None
            ex = smp.tile([1, L], F16, tag="ex", name="ex_l")
            ab = smp.tile([128, L], F16, tag="ab", name="ab_l")
            z0 = smp.tile([1, 1], F32, tag="z", name="z0")
            z1 = smp.tile([1, 1], F32, tag="z", name="z1")
            pu2 = outp.tile([128, 2 * HC], F32, tag="pu2")

            # half 0
            for ji, k in enumerate(korder0):
                emit_group(s, k, ups, 0, HL)
                if ji == 1:
                    emit_epilogue(s - 1)
                if ji == 4:
                    emit_drain(s - 2)
                if ji >= 2:
                    kk = korder0[ji - 2]
                    nc.tensor.matmul(nu_a[:, 0:HL], wnu_sb[:, kk:kk + 1],
                                     ups[:, kk, 0:HL],
                                     start=(ji == 2), stop=False)
            # half 1 (k=5 first so tanh(k=4) barely gates the nu tail)
            for ji, k in enumerate(korder1):
                emit_group(s, k, ups, HL, L)
                if ji == 0:
                    kk = korder0[4]
                    nc.tensor.matmul(nu_a[:, 0:HL], wnu_sb[:, kk:kk + 1],
                                     ups[:, kk, 0:HL], start=False, stop=False)
                if ji == 1:
                    kk = korder0[5]
                    nc.tensor.matmul(nu_a[:, 0:HL], wnu_sb[:, kk:kk + 1],
                                     ups[:, kk, 0:HL], start=False, stop=True)
                    # half-0 epilogue: unnormalized pooling into pu2[:, 0:6]
                    nc.scalar.activation(ex[:, 0:HL], nu_a[:, 0:HL], AF.Exp,
                                         accum_out=z0[:])
                    nc.gpsimd.partition_broadcast(ab[:, 0:HL], ex[:, 0:HL])
                    for h in range(HC):
                        trash = smp.tile([128, L], F16, tag="trash",
                                         name=f"tr0{h}")
                        nc.vector.scalar_tensor_tensor(
                            trash[:, 0:HL], xt_sl(s, h)[:, 0:HL], 1.0,
                            ab[:, 0:HL], ALU.mult, ALU.mult,
                            accum_out=pu2[:, h:h + 1],
                        )
                if ji >= 2:
                    kk = korder1[ji - 2]
                    if nu_b is None:
                        nu_b = nups.tile([1, L], F32, tag="nu", name="nu_b")
                    nc.tensor.matmul(nu_b[:, 0:HL], wnu_sb[:, kk:kk + 1],
                                     ups[:, kk, HL:L],
                                     start=(ji == 2), stop=False)
            for i, kk in enumerate([korder1[4], korder1[5]]):
                nc.tensor.matmul(nu_b[:, 0:HL], wnu_sb[:, kk:kk + 1],
                                 ups[:, kk, HL:L], start=False, stop=(i == 1))
            # half-1 epilogue: normalized pooling into pu2[:, 6:12]
            zs = smp.tile([1, 1], F32, tag="zs")
            rz = smp.tile([1, 1], F32, tag="rz", name="rz_l")
            rzb = smp.tile([128, 1], F32, tag="rzb", name="rzb_l")
            nc.scalar.activation(ex[:, HL:L], nu_b[:, 0:HL], AF.Exp,
                                 accum_out=z1[:])
            nc.vector.tensor_tensor(zs[:], z0[:], z1[:], ALU.add)
            nc.vector.reciprocal(rz[:], zs[:])
            nc.gpsimd.partition_broadcast(ab[:, HL:L], ex[:, HL:L])
            nc.gpsimd.partition_broadcast(rzb[:], rz[:])
            for h in range(HC):
                trash = smp.tile([128, L], F16, tag="trash", name=f"tr1{h}")
                nc.vector.scalar_tensor_tensor(
                    trash[:, 0:HL], xt_sl(s, h)[:, HL:L], rzb[:, 0:1],
                    ab[:, HL:L], ALU.mult, ALU.mult,
                    accum_out=pu2[:, HC + h:HC + h + 1],
                )
            # combine: pooled = pu_half0 * rz + pu_half1 (already * rz)
            pu_f = outp.tile([128, HC], F32, tag="pu", name="pu_last")
            nc.vector.scalar_tensor_tensor(
                pu_f[:], pu2[:, 0:HC], rzb[:, 0:1], pu2[:, HC:2 * HC],
                ALU.mult, ALU.add,
            )
            pu_t[s] = pu_f
            emit_drain(s - 1)
            emit_drain(s)

    nc.finalize()
    return nc


def kernel(hidden_states, W_fc, b_fc, W_nu, _trace=False, _trace_kwargs=None):
    from concourse.bass_utils import run_bass_kernel_spmd

    hs = np.ascontiguousarray(hidden_states, dtype=np.float32)
    W_fc = np.asarray(W_fc, np.float32)
    b_fc = np.asarray(b_fc, np.float32)
    W_nu = np.asarray(W_nu, np.float32)

    # W^T in [128, kstrip, h, m] layout:
    # [p, ks, h, m] = W_fc[ks*128+m, h*128+p]
    wth = np.ascontiguousarray(
        W_fc.T.reshape(HC, 128, HC, 128).transpose(1, 2, 0, 3)
        .astype(np.float16))
    wt0_host = np.ascontiguousarray(wth[:, 0:2])
    wt2_host = np.ascontiguousarray(wth[:, 2:3])
    wtr_host = np.ascontiguousarray(wth[:, 3:6])
    bfc_host = np.ascontiguousarray(b_fc.reshape(HC, 128).T, np.float32)
    wnu_host = np.ascontiguousarray(W_nu.reshape(HC, 128).T.astype(np.float16))

    in_maps = []
    for c in range(NCORES):
        # X^T in sample-major [128, (s c t)] layout so each per-sample DMA
        # is 128 contiguous 6KB descriptors:
        # [p, s, c, t] = X[s*512+t, c*128+p]
        xt = np.ascontiguousarray(
            hs[c * SPC:(c + 1) * SPC].reshape(TOK, H).T
            .reshape(HC, 128, SPC, L).transpose(1, 2, 0, 3)
            .reshape(128, SPC * HC * L).astype(np.float16))
        in_maps.append({"xt": xt, "wt0": wt0_host, "wt2": wt2_host,
                        "wtr": wtr_host, "bfc": bfc_host, "wnu": wnu_host})

    if "nc" not in _compiled:
        _compiled["nc"] = _build()
    res = run_bass_kernel_spmd(
        _compiled["nc"], in_maps, list(range(NCORES)),
        trace=_trace, **(_trace_kwargs or {}),
    )
    kernel.last_results = res
    out = np.concatenate([np.asarray(r["out"], np.float32) for r in res.results])
    return out


# revision 27
# speedup vs baseline: 1.0066x; 1.0066x over previous
"""AttentionProtoNet pooling kernel for 8x TRN2 NeuronCores.

reference (per sample of B=64, L=512, H=768):
    upsilon = tanh(hs @ W_fc.T + b_fc)        [L, H]
    nu      = upsilon @ W_nu                  [L]
    alphas  = softmax(nu)                     [L]
    pooled  = alphas @ hs                     [H]

Strategy: data-parallel over B (8 samples per core), everything on the wire
in fp16 (1 cycle/row on the PE like bf16, but with 10 mantissa bits, and a
single X^T copy feeds both the TensorEngine matmul and the VectorEngine
pooling). The PE runs back-to-back 512-row fp16 matmuls at its 216 ns
roofline cadence; each sample's nu/softmax/pooling epilogue is emitted
inside the NEXT sample's matmul stream so the PE never waits on ACT/DVE,
and the output drain (PE transpose -> copy -> DRAM) trails two samples
behind. DMA goes through the sync HW queue ordered so the PE starts as
early as possible (k0/k1 weights, first sample in halves, k2, remaining
weights, remaining samples). The last sample is processed in two 256-token
halves so most of its softmax/pooling chain overlaps its own matmuls.
"""

import sys

sys.path.insert(0, "/opt/trn_rl_repo")

import numpy as np

B, L, H = 64, 512, 768
NCORES = 8
SPC = B // NCORES            # samples per core
TOK = SPC * L                # tokens per core
HC = H // 128                # 128-partition chunks of H
HL = L // 2                  # token half for the last sample
WARMUP_MM = 9                # junk matmuls: p-state + HAM ramp during DMA

_compiled = {}


def _build():
    import concourse.bass as bass
    import concourse.bacc as bacc
    import concourse.tile as tile
    from concourse import mybir
    from concourse.masks import make_identity

    F32 = mybir.dt.float32
    F16 = mybir.dt.float16
    AF = mybir.ActivationFunctionType
    ALU = mybir.AluOpType

    nc = bacc.Bacc(None, target_bir_lowering=False)

    xt_d = nc.dram_tensor("xt", [128, SPC * HC * L], F16, kind="ExternalInput")
    # weights packed k-strip-major so each weight DMA is 128 large
    # contiguous descriptors
    wt0_d = nc.dram_tensor("wt0", [128, 2, HC, 128], F16, kind="ExternalInput")
    wt2_d = nc.dram_tensor("wt2", [128, 1, HC, 128], F16, kind="ExternalInput")
    wtr_d = nc.dram_tensor("wtr", [128, 3, HC, 128], F16, kind="ExternalInput")
    bfc_d = nc.dram_tensor("bfc", [128, HC], F32, kind="ExternalInput")
    wnu_d = nc.dram_tensor("wnu", [128, HC], F16, kind="ExternalInput")
    out_d = nc.dram_tensor("out", [SPC, H], F32, kind="ExternalOutput")

    with tile.TileContext(nc) as tc:
        with tc.tile_pool(name="xp", bufs=1) as xp, \
             tc.tile_pool(name="wp", bufs=1) as wp, \
             tc.tile_pool(name="cst", bufs=1) as cst, \
             tc.tile_pool(name="ups", bufs=2) as upsp, \
             tc.tile_pool(name="sm", bufs=2) as smp, \
             tc.tile_pool(name="outp", bufs=2) as outp, \
             tc.tile_pool(name="mmps", bufs=4, space="PSUM") as mmps, \
             tc.tile_pool(name="nups", bufs=2, space="PSUM") as nups, \
             tc.tile_pool(name="tps", bufs=2, space="PSUM") as tps:

            # ---- PE warmup: junk matmuls with no DMA dependency ramp the
            # p-state and the HAM activity window while the first tiles
            # stream in.
            wu_sb = cst.tile([128, L], F16)
            nc.vector.memset(wu_sb[:], 1.0)
            wu_ps = tps.tile([128, L], F32, tag="tp", name="wu_ps")
            for i in range(WARMUP_MM):
                nc.tensor.matmul(wu_ps[:], wu_sb[:, 0:128], wu_sb[:],
                                 start=(i == 0), stop=(i == WARMUP_MM - 1))

            # ---- DMA: tiny constants on the gpsimd direct queue; weights
            # and X^T through the sync HW queue, interleaved so the first
            # sample's matmuls start as early as possible.
            bfc_sb = cst.tile([128, HC], F32)
            wnu_sb = cst.tile([128, HC], F16)
            wt_sb = wp.tile([128, HC, HC, 128], F16)   # [p, kstrip, h, m]
            xt_sb = xp.tile([128, SPC * HC * L], F16)
            ident = cst.tile([128, 128], F32)

            nc.gpsimd.dma_start(bfc_sb[:], bfc_d[:])
            nc.gpsimd.dma_start(wnu_sb[:], wnu_d[:])

            def xt_sl(s, h):
                return xt_sb[:, (s * HC + h) * L:(s * HC + h + 1) * L]

            nc.sync.dma_start(wt_sb[:, 0:2, :, :], wt0_d[:])
            nc.sync.dma_start(xt_sb[:, 0:3 * L], xt_d[:, 0:3 * L])
            nc.sync.dma_start(xt_sb[:, 3 * L:HC * L], xt_d[:, 3 * L:HC * L])
            nc.sync.dma_start(wt_sb[:, 2:3, :, :], wt2_d[:])
            nc.sync.dma_start(wt_sb[:, 3:6, :, :], wtr_d[:])
            for s in range(1, SPC):
                nc.sync.dma_start(xt_sb[:, s * HC * L:(s + 1) * HC * L],
                                  xt_d[:, s * HC * L:(s + 1) * HC * L])
            make_identity(nc, ident[:])

            # ---- per-sample state carried to later emission points
            ups_t = [None] * SPC
            # all samples' pooled vectors gather into one [128, 48] tile,
            # drained by a single transpose + copy + DMA at the end
            pucat = outp.tile([128, SPC * HC], F32, tag="pucat")

            def emit_group(s, k, ups, c0, c1):
                """one k-chunk matmul group + tanh for tokens [c0, c1)."""
                ps = mmps.tile([128, L], F32, tag="mm")
                w = c1 - c0
                for h in range(HC):
                    nc.tensor.matmul(
                        ps[:, 0:w],
                        wt_sb[:, k, h, :],
                        xt_sl(s, h)[:, c0:c1],
                        start=(h == 0),
                        stop=(h == HC - 1),
                    )
                nc.scalar.activation(
                    ups[:, k, c0:c1], ps[:, 0:w], AF.Tanh,
                    bias=bfc_sb[:, k:k + 1],
                )

            def emit_epilogue(s):
                """nu + softmax + pooling for sample s, emitted inside a
                later sample's matmul stream where all inputs are done."""
                ups = ups_t[s]
                nu = nups.tile([1, L], F32, tag="nu", name="nu_p")
                for k in range(HC):
                    nc.tensor.matmul(
                        nu[:], wnu_sb[:, k:k + 1], ups[:, k, :],
                        start=(k == 0), stop=(k == HC - 1),
                    )
                # nu is small enough that exp() needs no max subtraction
                ex = smp.tile([1, L], F16, tag="ex")
                z = smp.tile([1, 1], F32, tag="z")
                rz = smp.tile([1, 1], F32, tag="rz")
                nc.scalar.activation(ex[:], nu[:], AF.Exp, accum_out=z[:])
                nc.vector.reciprocal(rz[:], z[:])
                ab = smp.tile([128, L], F16, tag="ab")
                nc.gpsimd.partition_broadcast(ab[:], ex[:])
                rzb = smp.tile([128, 1], F32, tag="rzb")
                nc.gpsimd.partition_broadcast(rzb[:], rz[:])
                # weighted-sum pooling on the VectorEngine; the 1/Z
                # normalization rides the STT per-partition scalar
                for h in range(HC):
                    trash = smp.tile([128, L], F16, tag="trash")
                    nc.vector.scalar_tensor_tensor(
                        trash[:], xt_sl(s, h), rzb[:, 0:1], ab[:],
                        ALU.mult, ALU.mult,
                        accum_out=pucat[:, s * HC + h:s * HC + h + 1],
                    )

            # ---- samples 0..6: full-width pipeline
            for s in range(SPC - 1):
                ups = upsp.tile([128, HC, L], F16, tag="ups")
                ups_t[s] = ups
                for ji in range(HC):
                    emit_group(s, ji, ups, 0, L)
                    if s > 0 and ji == 1:
                        emit_epilogue(s - 1)

            # ---- last sample: two 256-token halves so the softmax/pool
            # chain of half 0 overlaps half 1's matmuls, and only a short
            # chain trails the final matmul
            s = SPC - 1
            ups = upsp.tile([128, HC, L], F16, tag="ups", name="ups_last")
            ups_t[s] = ups
            korder0 = list(range(HC))
            korder1 = [5, 0, 1, 2, 3, 4]
            nu_a = nups.tile([1, L], F32, tag="nu", name="nu_a")
            nu_b = None
            ex = smp.tile([1, L], F16, tag="ex", name="ex_l")
            ab = smp.tile([128, L], F16, tag="ab", name="ab_l")
            z0 = smp.tile([1, 1], F32, tag="z", name="z0")
            z1 = smp.tile([1, 1], F32, tag="z", name="z1")
            pu2 = outp.tile([128, 2 * HC], F32, tag="pu2")

            # half 0
            for ji, k in enumerate(korder0):
                emit_group(s, k, ups, 0, HL)
                if ji == 1:
                    emit_epilogue(s - 1)
                if ji >= 2:
                    kk = korder0[ji - 2]
                    nc.tensor.matmul(nu_a[:, 0:HL], wnu_sb[:, kk:kk + 1],
                                     ups[:, kk, 0:HL],
                                     start=(ji == 2), stop=False)
            # half 1 (k=5 first so tanh(k=4) barely gates the nu tail)
            for ji, k in enumerate(korder1):
                emit_group(s, k, ups, HL, L)
                if ji == 0:
                    kk = korder0[4]
                    nc.tensor.matmul(nu_a[:, 0:HL], wnu_sb[:, kk:kk + 1],
                                     ups[:, kk, 0:HL], start=False, stop=False)
                if ji == 1:
                    kk = korder0[5]
                    nc.tensor.matmul(nu_a[:, 0:HL], wnu_sb[:, kk:kk + 1],
                                     ups[:, kk, 0:HL], start=False, stop=True)
                    # half-0 epilogue: unnormalized pooling into pu2[:, 0:6]
                    nc.scalar.activation(ex[:, 0:HL], nu_a[:, 0:HL], AF.Exp,
                                         accum_out=z0[:])
                    nc.gpsimd.partition_broadcast(ab[:, 0:HL], ex[:, 0:HL])
                    for h in range(HC):
                        trash = smp.tile([128, L], F16, tag="trash",
                                         name=f"tr0{h}")
                        nc.vector.scalar_tensor_tensor(
                            trash[:, 0:HL], xt_sl(s, h)[:, 0:HL], 1.0,
                            ab[:, 0:HL], ALU.mult, ALU.mult,
                            accum_out=pu2[:, h:h + 1],
                        )
                if ji >= 2:
                    kk = korder1[ji - 2]
                    if nu_b is None:
                        nu_b = nups.tile([1, L], F32, tag="nu", name="nu_b")
                    nc.tensor.matmul(nu_b[:, 0:HL], wnu_sb[:, kk:kk + 1],
                                     ups[:, kk, HL:L],
                                     start=(ji == 2), stop=False)
            for i, kk in enumerate([korder1[4], korder1[5]]):
                nc.tensor.matmul(nu_b[:, 0:HL], wnu_sb[:, kk:kk + 1],
                                 ups[:, kk, HL:L], start=False, stop=(i == 1))
            # half-1 epilogue: normalized pooling into pu2[:, 6:12]
            zs = smp.tile([1, 1], F32, tag="zs")
            rz = smp.tile([1, 1], F32, tag="rz", name="rz_l")
            rzb = smp.tile([128, 1], F32, tag="rzb", name="rzb_l")
            nc.scalar.activation(ex[:, HL:L], nu_b[:, 0:HL], AF.Exp,
                                 accum_out=z1[:])
            nc.vector.tensor_tensor(zs[:], z0[:], z1[:], ALU.add)
            nc.vector.reciprocal(rz[:], zs[:])
            nc.gpsimd.partition_broadcast(ab[:, HL:L], ex[:, HL:L])
            nc.gpsimd.partition_broadcast(rzb[:], rz[:])
            for h in range(HC):
                trash = smp.tile([128, L], F16, tag="trash", name=f"tr1{h}")
                nc.vector.scalar_tensor_tensor(
                    trash[:, 0:HL], xt_sl(s, h)[:, HL:L], rzb[:, 0:1],
                    ab[:, HL:L], ALU.mult, ALU.mult,
                    accum_out=pu2[:, HC + h:HC + h + 1],
                )
            # combine: pooled = pu_half0 * rz + pu_half1 (already * rz)
            nc.vector.scalar_tensor_tensor(
                pucat[:, s * HC:(s + 1) * HC], pu2[:, 0:HC], rzb[:, 0:1],
                pu2[:, HC:2 * HC], ALU.mult, ALU.add,
            )
            # single gathered drain: [128, 48] -> [48, 128] -> DRAM
            tp_all = tps.tile([SPC * HC, 128], F32, tag="tp", name="tp_all")
            nc.tensor.transpose(tp_all[:], pucat[:], ident[:])
            orow = outp.tile([SPC * HC, 128], F32, tag="orow")
            nc.scalar.copy(orow[:], tp_all[:])
            nc.sync.dma_start(
                out_d[:, :].rearrange("s (c p) -> (s c) p", p=128),
                orow[:],
            )

    nc.finalize()
    return nc


def kernel(hidden_states, W_fc, b_fc, W_nu, _trace=False, _trace_kwargs=None):
    from concourse.bass_utils import run_bass_kernel_spmd

    hs = np.ascontiguousarray(hidden_states, dtype=np.float32)
    W_fc = np.asarray(W_fc, np.float32)
    b_fc = np.asarray(b_fc, np.float32)
    W_nu = np.asarray(W_nu, np.float32)

    # W^T in [128, kstrip, h, m] layout:
    # [p, ks, h, m] = W_fc[ks*128+m, h*128+p]
    wth = np.ascontiguousarray(
        W_fc.T.reshape(HC, 128, HC, 128).transpose(1, 2, 0, 3)
        .astype(np.float16))
    wt0_host = np.ascontiguousarray(wth[:, 0:2])
    wt2_host = np.ascontiguousarray(wth[:, 2:3])
    wtr_host = np.ascontiguousarray(wth[:, 3:6])
    bfc_host = np.ascontiguousarray(b_fc.reshape(HC, 128).T, np.float32)
    wnu_host = np.ascontiguousarray(W_nu.reshape(HC, 128).T.astype(np.float16))

    in_maps = []
    for c in range(NCORES):
        # X^T in sample-major [128, (s c t)] layout so each per-sample DMA
        # is 128 contiguous 6KB descriptors:
        # [p, s, c, t] = X[s*512+t, c*128+p]
        xt = np.ascontiguousarray(
            hs[c * SPC:(c + 1) * SPC].reshape(TOK, H).T
            .reshape(HC, 128, SPC, L).transpose(1, 2, 0, 3)
            .reshape(128, SPC * HC * L).astype(np.float16))
        in_maps.append({"xt": xt, "wt0": wt0_host, "wt2": wt2_host,
                        "wtr": wtr_host, "bfc": bfc_host, "wnu": wnu_host})

    if "nc" not in _compiled:
        _compiled["nc"] = _build()
    res = run_bass_kernel_spmd(
        _compiled["nc"], in_maps, list(range(NCORES)),
        trace=_trace, **(_trace_kwargs or {}),
    )
    kernel.last_results = res
    out = np.concatenate([np.asarray(r["out"], np.float32) for r in res.results])
    return out


# revision 31
# speedup vs baseline: 1.0138x; 1.0072x over previous
"""AttentionProtoNet pooling kernel for 8x TRN2 NeuronCores.

reference (per sample of B=64, L=512, H=768):
    upsilon = tanh(hs @ W_fc.T + b_fc)        [L, H]
    nu      = upsilon @ W_nu                  [L]
    alphas  = softmax(nu)                     [L]
    pooled  = alphas @ hs                     [H]

Strategy: data-parallel over B (8 samples per core), everything on the wire
in fp16 (1 cycle/row on the PE like bf16, but with 10 mantissa bits, and a
single X^T copy feeds both the TensorEngine matmul and the VectorEngine
pooling). The PE runs back-to-back 512-row fp16 matmuls at its 216 ns
roofline cadence; each sample's nu/softmax/pooling epilogue is emitted
inside the NEXT sample's matmul stream so the PE never waits on ACT/DVE,
and the output drain (PE transpose -> copy -> DRAM) trails two samples
behind. DMA goes through the sync HW queue ordered so the PE starts as
early as possible (k0/k1 weights, first sample in halves, k2, remaining
weights, remaining samples). The last sample is processed in two 256-token
halves so most of its softmax/pooling chain overlaps its own matmuls.
"""

import sys

sys.path.insert(0, "/opt/trn_rl_repo")

import numpy as np

B, L, H = 64, 512, 768
NCORES = 8
SPC = B // NCORES            # samples per core
TOK = SPC * L                # tokens per core
HC = H // 128                # 128-partition chunks of H
HL = L // 2                  # token half for the last sample
WARMUP_MM = 9                # junk matmuls: p-state + HAM ramp during DMA

_compiled = {}


def _build():
    import concourse.bass as bass
    import concourse.bacc as bacc
    import concourse.tile as tile
    from concourse import mybir
    from concourse.masks import make_identity

    F32 = mybir.dt.float32
    F16 = mybir.dt.float16
    AF = mybir.ActivationFunctionType
    ALU = mybir.AluOpType

    nc = bacc.Bacc(None, target_bir_lowering=False)

    xt_d = nc.dram_tensor("xt", [128, SPC * HC * L], F16, kind="ExternalInput")
    # weights packed k-strip-major so each weight DMA is 128 large
    # contiguous descriptors
    wt0_d = nc.dram_tensor("wt0", [128, 1, HC, 128], F16, kind="ExternalInput")
    wt1_d = nc.dram_tensor("wt1", [128, 1, HC, 128], F16, kind="ExternalInput")
    wt2_d = nc.dram_tensor("wt2", [128, 1, HC, 128], F16, kind="ExternalInput")
    wtr_d = nc.dram_tensor("wtr", [128, 3, HC, 128], F16, kind="ExternalInput")
    bfc_d = nc.dram_tensor("bfc", [128, HC], F32, kind="ExternalInput")
    wnu_d = nc.dram_tensor("wnu", [128, HC], F16, kind="ExternalInput")
    out_d = nc.dram_tensor("out", [SPC, H], F32, kind="ExternalOutput")

    with tile.TileContext(nc) as tc:
        with tc.tile_pool(name="xp", bufs=1) as xp, \
             tc.tile_pool(name="wp", bufs=1) as wp, \
             tc.tile_pool(name="cst", bufs=1) as cst, \
             tc.tile_pool(name="ups", bufs=2) as upsp, \
             tc.tile_pool(name="sm", bufs=2) as smp, \
             tc.tile_pool(name="outp", bufs=2) as outp, \
             tc.tile_pool(name="mmps", bufs=4, space="PSUM") as mmps, \
             tc.tile_pool(name="nups", bufs=2, space="PSUM") as nups, \
             tc.tile_pool(name="tps", bufs=2, space="PSUM") as tps:

            # ---- PE warmup: junk matmuls with no DMA dependency ramp the
            # p-state and the HAM activity window while the first tiles
            # stream in.
            wu_sb = cst.tile([128, L], F16)
            nc.vector.memset(wu_sb[:], 1.0)
            wu_ps = tps.tile([128, L], F32, tag="tp", name="wu_ps")
            for i in range(WARMUP_MM):
                nc.tensor.matmul(wu_ps[:], wu_sb[:, 0:128], wu_sb[:],
                                 start=(i == 0), stop=(i == WARMUP_MM - 1))

            # ---- DMA: tiny constants on the gpsimd direct queue; weights
            # and X^T through the sync HW queue, interleaved so the first
            # sample's matmuls start as early as possible.
            bfc_sb = cst.tile([128, HC], F32)
            wnu_sb = cst.tile([128, HC], F16)
            wt_sb = wp.tile([128, HC, HC, 128], F16)   # [p, kstrip, h, m]
            xt_sb = xp.tile([128, SPC * HC * L], F16)
            ident = cst.tile([128, 128], F32)

            nc.gpsimd.dma_start(bfc_sb[:], bfc_d[:])
            nc.gpsimd.dma_start(wnu_sb[:], wnu_d[:])

            def xt_sl(s, h):
                return xt_sb[:, (s * HC + h) * L:(s * HC + h + 1) * L]

            nc.sync.dma_start(wt_sb[:, 0:1, :, :], wt0_d[:])
            nc.sync.dma_start(xt_sb[:, 0:3 * L], xt_d[:, 0:3 * L])
            nc.sync.dma_start(xt_sb[:, 3 * L:HC * L], xt_d[:, 3 * L:HC * L])
            nc.sync.dma_start(wt_sb[:, 1:2, :, :], wt1_d[:])
            nc.sync.dma_start(wt_sb[:, 2:3, :, :], wt2_d[:])
            nc.sync.dma_start(wt_sb[:, 3:6, :, :], wtr_d[:])
            for s in range(1, SPC):
                nc.sync.dma_start(xt_sb[:, s * HC * L:(s + 1) * HC * L],
                                  xt_d[:, s * HC * L:(s + 1) * HC * L])
            make_identity(nc, ident[:])

            # ---- per-sample state carried to later emission points
            ups_t = [None] * SPC
            # all samples' pooled vectors gather into one [128, 48] tile,
            # drained by a single transpose + copy + DMA at the end
            pucat = outp.tile([128, SPC * HC], F32, tag="pucat")

            def emit_group(s, k, ups, c0, c1):
                """one k-chunk matmul group + tanh for tokens [c0, c1)."""
                ps = mmps.tile([128, L], F32, tag="mm")
                w = c1 - c0
                for h in range(HC):
                    nc.tensor.matmul(
                        ps[:, 0:w],
                        wt_sb[:, k, h, :],
                        xt_sl(s, h)[:, c0:c1],
                        start=(h == 0),
                        stop=(h == HC - 1),
                    )
                nc.scalar.activation(
                    ups[:, k, c0:c1], ps[:, 0:w], AF.Tanh,
                    bias=bfc_sb[:, k:k + 1],
                )

            def emit_epilogue(s):
                """nu + softmax + pooling for sample s, emitted inside a
                later sample's matmul stream where all inputs are done."""
                ups = ups_t[s]
                nu = nups.tile([1, L], F32, tag="nu", name="nu_p")
                for k in range(HC):
                    nc.tensor.matmul(
                        nu[:], wnu_sb[:, k:k + 1], ups[:, k, :],
                        start=(k == 0), stop=(k == HC - 1),
                    )
                # nu is small enough that exp() needs no max subtraction
                ex = smp.tile([1, L], F16, tag="ex")
                z = smp.tile([1, 1], F32, tag="z")
                rz = smp.tile([1, 1], F32, tag="rz")
                nc.scalar.activation(ex[:], nu[:], AF.Exp, accum_out=z[:])
                nc.vector.reciprocal(rz[:], z[:])
                ab = smp.tile([128, L], F16, tag="ab")
                nc.gpsimd.partition_broadcast(ab[:], ex[:])
                rzb = smp.tile([128, 1], F32, tag="rzb")
                nc.gpsimd.partition_broadcast(rzb[:], rz[:])
                # weighted-sum pooling on the VectorEngine; the 1/Z
                # normalization rides the STT per-partition scalar
                for h in range(HC):
                    trash = smp.tile([128, L], F16, tag="trash")
                    nc.vector.scalar_tensor_tensor(
                        trash[:], xt_sl(s, h), rzb[:, 0:1], ab[:],
                        ALU.mult, ALU.mult,
                        accum_out=pucat[:, s * HC + h:s * HC + h + 1],
                    )

            # ---- samples 0..6: full-width pipeline
            for s in range(SPC - 1):
                ups = upsp.tile([128, HC, L], F16, tag="ups")
                ups_t[s] = ups
                for ji in range(HC):
                    emit_group(s, ji, ups, 0, L)
                    if s > 0 and ji == 1:
                        emit_epilogue(s - 1)

            # ---- last sample: two 256-token halves so the softmax/pool
            # chain of half 0 overlaps half 1's matmuls, and only a short
            # chain trails the final matmul
            s = SPC - 1
            ups = upsp.tile([128, HC, L], F16, tag="ups", name="ups_last")
            ups_t[s] = ups
            korder0 = list(range(HC))
            korder1 = [5, 0, 1, 2, 3, 4]
            nu_a = nups.tile([1, L], F32, tag="nu", name="nu_a")
            nu_b = None
            ex = smp.tile([1, L], F16, tag="ex", name="ex_l")
            ab = smp.tile([128, L], F16, tag="ab", name="ab_l")
            z0 = smp.tile([1, 1], F32, tag="z", name="z0")
            z1 = smp.tile([1, 1], F32, tag="z", name="z1")
            pu2 = outp.tile([128, 2 * HC], F32, tag="pu2")

            # half 0
            for ji, k in enumerate(korder0):
                emit_group(s, k, ups, 0, HL)
                if ji == 1:
                    emit_epilogue(s - 1)
                if ji >= 2:
                    kk = korder0[ji - 2]
                    nc.tensor.matmul(nu_a[:, 0:HL], wnu_sb[:, kk:kk + 1],
                                     ups[:, kk, 0:HL],
                                     start=(ji == 2), stop=False)
            # half 1 (k=5 first so tanh(k=4) barely gates the nu tail)
            for ji, k in enumerate(korder1):
                emit_group(s, k, ups, HL, L)
                if ji == 0:
                    kk = korder0[4]
                    nc.tensor.matmul(nu_a[:, 0:HL], wnu_sb[:, kk:kk + 1],
                                     ups[:, kk, 0:HL], start=False, stop=False)
                if ji == 1:
                    kk = korder0[5]
                    nc.tensor.matmul(nu_a[:, 0:HL], wnu_sb[:, kk:kk + 1],
                                     ups[:, kk, 0:HL], start=False, stop=True)
                    # half-0 epilogue: unnormalized pooling into pu2[:, 0:6]
                    nc.scalar.activation(ex[:, 0:HL], nu_a[:, 0:HL], AF.Exp,
                                         accum_out=z0[:])
                    nc.gpsimd.partition_broadcast(ab[:, 0:HL], ex[:, 0:HL])
                    for h in range(HC):
                        trash = smp.tile([128, L], F16, tag="trash",
                                         name=f"tr0{h}")
                        nc.vector.scalar_tensor_tensor(
                            trash[:, 0:HL], xt_sl(s, h)[:, 0:HL], 1.0,
                            ab[:, 0:HL], ALU.mult, ALU.mult,
                            accum_out=pu2[:, h:h + 1],
                        )
                if ji >= 2:
                    kk = korder1[ji - 2]
                    if nu_b is None:
                        nu_b = nups.tile([1, L], F32, tag="nu", name="nu_b")
                    nc.tensor.matmul(nu_b[:, 0:HL], wnu_sb[:, kk:kk + 1],
                                     ups[:, kk, HL:L],
                                     start=(ji == 2), stop=False)
            for i, kk in enumerate([korder1[4], korder1[5]]):
                nc.tensor.matmul(nu_b[:, 0:HL], wnu_sb[:, kk:kk + 1],
                                 ups[:, kk, HL:L], start=False, stop=(i == 1))
            # half-1 epilogue: normalized pooling into pu2[:, 6:12]
            zs = smp.tile([1, 1], F32, tag="zs")
            rz = smp.tile([1, 1], F32, tag="rz", name="rz_l")
            rzb = smp.tile([128, 1], F32, tag="rzb", name="rzb_l")
            nc.scalar.activation(ex[:, HL:L], nu_b[:, 0:HL], AF.Exp,
                                 accum_out=z1[:])
            nc.vector.tensor_tensor(zs[:], z0[:], z1[:], ALU.add)
            nc.vector.reciprocal(rz[:], zs[:])
            nc.gpsimd.partition_broadcast(ab[:, HL:L], ex[:, HL:L])
            nc.gpsimd.partition_broadcast(rzb[:], rz[:])
            for h in range(HC):
                trash = smp.tile([128, L], F16, tag="trash", name=f"tr1{h}")
                nc.vector.scalar_tensor_tensor(
                    trash[:, 0:HL], xt_sl(s, h)[:, HL:L], rzb[:, 0:1],
                    ab[:, HL:L], ALU.mult, ALU.mult,
                    accum_out=pu2[:, HC + h:HC + h + 1],
                )
            # combine: pooled = pu_half0 * rz + pu_half1 (already * rz)
            nc.vector.scalar_tensor_tensor(
                pucat[:, s * HC:(s + 1) * HC], pu2[:, 0:HC], rzb[:, 0:1],
                pu2[:, HC:2 * HC], ALU.mult, ALU.add,
            )
            # single gathered drain: [128, 48] -> [48, 128] -> DRAM
            tp_all = tps.tile([SPC * HC, 128], F32, tag="tp", name="tp_all")
            nc.tensor.transpose(tp_all[:], pucat[:], ident[:])
            orow = outp.tile([SPC * HC, 128], F32, tag="orow")
            nc.scalar.copy(orow[:], tp_all[:])
            nc.sync.dma_start(
                out_d[:, :].rearrange("s (c p) -> (s c) p", p=128),
                orow[:],
            )

    nc.finalize()
    return nc


def kernel(hidden_states, W_fc, b_fc, W_nu, _trace=False, _trace_kwargs=None):
    from concourse.bass_utils import run_bass_kernel_spmd

    hs = np.ascontiguousarray(hidden_states, dtype=np.float32)
    W_fc = np.asarray(W_fc, np.float32)
    b_fc = np.asarray(b_fc, np.float32)
    W_nu = np.asarray(W_nu, np.float32)

    # W^T in [128, kstrip, h, m] layout:
    # [p, ks, h, m] = W_fc[ks*128+m, h*128+p]
    wth = np.ascontiguousarray(
        W_fc.T.reshape(HC, 128, HC, 128).transpose(1, 2, 0, 3)
        .astype(np.float16))
    wt0_host = np.ascontiguousarray(wth[:, 0:1])
    wt1_host = np.ascontiguousarray(wth[:, 1:2])
    wt2_host = np.ascontiguousarray(wth[:, 2:3])
    wtr_host = np.ascontiguousarray(wth[:, 3:6])
    bfc_host = np.ascontiguousarray(b_fc.reshape(HC, 128).T, np.float32)
    wnu_host = np.ascontiguousarray(W_nu.reshape(HC, 128).T.astype(np.float16))

    in_maps = []
    for c in range(NCORES):
        # X^T in sample-major [128, (s c t)] layout so each per-sample DMA
        # is 128 contiguous 6KB descriptors:
        # [p, s, c, t] = X[s*512+t, c*128+p]
        xt = np.ascontiguousarray(
            hs[c * SPC:(c + 1) * SPC].reshape(TOK, H).T
            .reshape(HC, 128, SPC, L).transpose(1, 2, 0, 3)
            .reshape(128, SPC * HC * L).astype(np.float16))
        in_maps.append({"xt": xt, "wt0": wt0_host, "wt1": wt1_host,
                        "wt2": wt2_host, "wtr": wtr_host,
                        "bfc": bfc_host, "wnu": wnu_host})

    if "nc" not in _compiled:
        _compiled["nc"] = _build()
    res = run_bass_kernel_spmd(
        _compiled["nc"], in_maps, list(range(NCORES)),
        trace=_trace, **(_trace_kwargs or {}),
    )
    kernel.last_results = res
    out = np.concatenate([np.asarray(r["out"], np.float32) for r in res.results])
    return out
